# revision 6
# baseline (speedup 1.0000x reference)
"""Bass/Tile Trainium2 kernel for nn_CrossAttention (8 NeuronCores).

Sharding: data-parallel over batch N=4, and each sample's query positions
are split across 2 cores (rows 0..23 / 24..47 of the 48x64 image, plus a
4-row halo so the conv stack after attention needs no communication).

Per-core pipeline (uniform across cores; all asymmetry handled on host):
  valT = (w_value @ v)^T                       [kpos, d] tiles
  scores[k, q] = k^T q                         PE, fp32r
  e = exp(scores / sqrt(384))                  ACT, PSUM->SBUF (fp32r out)
  pv[d, q]  = val @ e                          PE accumulation over k-tiles
  cs[q]     = ones^T e  (broadcast to 128p)    PE accumulation over k-tiles
  p = pv / cs                                  DVE (reciprocal + mul)
  v2 = v + lrelu(conv3x3(p))                   PE + DVE
  f  = lrelu(dconv3x3_d2(v2))                  PE + DVE
  out = v2 + lrelu(conv3x3(f))                 PE + DVE

fp32r note: the PE runs fp32r (full-rate single-pass fp32) only when every
matmul operand was *produced* as fp32r by a compute instruction, so DMA'd
tensors are bounced through a rounding copy and on-chip producers (ACT exp,
DVE image writes) emit fp32r directly.
"""

import math

import numpy as np

import concourse.bass as bass
import concourse.mybir as mybir
import concourse.tile as tile
from concourse import bacc
from concourse.bass_utils import run_bass_kernel_spmd

F32 = mybir.dt.float32
F32R = mybir.dt.float32r
AF = mybir.ActivationFunctionType

# Problem constants (hardcoded per the contract)
N_BATCH = 4
C = 384            # kq channels
D = 128            # hidden
H, W = 48, 64
HW = H * W         # 3072
CT = C // 128      # 3 contraction tiles
KT = HW // 128     # 24 key-position tiles
RQ = 28            # local rows per core (24 output + 4 halo)
Q = RQ * W         # 1792 query positions per core
RB = 7             # image rows per block
NB = RQ // RB      # 4 blocks
QB = RB * W        # 448 block free size
SCALE = 1.0 / math.sqrt(C)
LRELU_SLOPE = 0.2

USE_F32R = True    # run the big matmuls in fp32r (full-rate fp32)

MMDT = F32R if USE_F32R else F32


def _r3(ap):
    """[128, QB] -> [128, RB, W] view."""
    return ap.rearrange("p (r w) -> p r w", w=W)


def _f32(ap):
    """Read an fp32r tile as plain fp32 (same bits) for DVE/ACT consumers."""
    return ap.bitcast(F32) if ap.dtype == F32R else ap


def build_nc():
    nc = bacc.Bacc(None, target_bir_lowering=False)

    kk = nc.dram_tensor("kk", [128, KT, CT, 128], F32, kind="ExternalInput")
    qq = nc.dram_tensor("qq", [128, NB, CT, QB], F32, kind="ExternalInput")
    vv = nc.dram_tensor("vv", [128, HW], F32, kind="ExternalInput")
    vloc = nc.dram_tensor("vloc", [128, RQ, W], F32, kind="ExternalInput")
    wv = nc.dram_tensor("wv", [128, 128], F32, kind="ExternalInput")
    wo = nc.dram_tensor("wo", [128, 9, 128], F32, kind="ExternalInput")
    wf1 = nc.dram_tensor("wf1", [128, 9, 128], F32, kind="ExternalInput")
    wf2 = nc.dram_tensor("wf2", [128, 9, 128], F32, kind="ExternalInput")
    out_d = nc.dram_tensor("out", [128, RQ, W], F32, kind="ExternalOutput")

    with tile.TileContext(nc) as tc:
        with (
            tc.tile_pool(name="io", bufs=1) as io,
            tc.tile_pool(name="stage", bufs=3) as stage,
            tc.tile_pool(name="work", bufs=4) as work,
            tc.tile_pool(name="psmm", bufs=4, space="PSUM") as psmm,
            tc.tile_pool(name="pspv", bufs=1, space="PSUM") as pspv,
            tc.tile_pool(name="pscs", bufs=1, space="PSUM") as pscs,
            tc.tile_pool(name="pscv", bufs=2, space="PSUM") as pscv,
        ):
            # persistent SBUF tensors (matmul operands in MMDT)
            k_t = [io.tile([128, CT, 128], MMDT, tag=f"k{kt}", name=f"k{kt}") for kt in range(KT)]
            q_t = [io.tile([128, CT, QB], MMDT, tag=f"q{b}", name=f"q{b}") for b in range(NB)]
            v_sb = io.tile([128, HW], F32, tag="v", name="v")
            vl_sb = io.tile([128, RQ, W], F32, tag="vl", name="vl")
            wv_sb = io.tile([128, 128], F32, tag="wv", name="wv")
            wo_sb = io.tile([128, 9, 128], MMDT, tag="wo", name="wo")
            wf1_sb = io.tile([128, 9, 128], MMDT, tag="wf1", name="wf1")
            wf2_sb = io.tile([128, 9, 128], MMDT, tag="wf2", name="wf2")
            ones_sb = io.tile([128, 128], MMDT, tag="ones", name="ones")
            valT = [io.tile([128, 512], MMDT, tag=f"vt{j}", name=f"vt{j}") for j in range(KT // 4)]
            p_img = io.tile([128, RQ + 2, W + 2], MMDT, tag="p_img", name="p_img")
            v2_img = io.tile([128, RQ + 4, W + 4], MMDT, tag="v2_img", name="v2_img")
            f_img = io.tile([128, RQ + 2, W + 2], MMDT, tag="f_img", name="f_img")
            out_sb = io.tile([128, RQ, W], F32, tag="out_sb", name="out_sb")

            def load(dst, src_ap, nbytes_tag):
                """DMA to a staging f32 tile, then round-copy into dst (DVE)."""
                if dst.dtype == F32:
                    nc.sync.dma_start(dst[:], src_ap)
                    return
                st = stage.tile(list(dst.shape), F32, tag=nbytes_tag, name=nbytes_tag)
                nc.sync.dma_start(st[:], src_ap)
                nc.vector.tensor_copy(dst[:], st[:])

            # input DMAs (+ fp32r rounding bounce where needed); q block 0
            # first so the first scores matmul can start ASAP
            load(q_t[0], qq[:, 0, :, :], "stq")
            for kt in range(KT):
                load(k_t[kt], kk[:, kt, :, :], "stk")
            for b in range(1, NB):
                load(q_t[b], qq[:, b, :, :], "stq")
            nc.sync.dma_start(v_sb[:], vv[:])
            nc.sync.dma_start(vl_sb[:], vloc[:])
            nc.sync.dma_start(wv_sb[:], wv[:])
            load(wo_sb, wo[:], "stw")
            load(wf1_sb, wf1[:], "stw")
            load(wf2_sb, wf2[:], "stw")

            # constants / zero borders, produced by compute ops so the
            # fp32r verifier sees rounded producers
            zer = io.tile([128, (RQ + 4) * (W + 4)], F32, tag="zer", name="zer")
            nc.vector.memset(zer[:], 0.0)
            one = io.tile([128, 128], F32, tag="one", name="one")
            nc.vector.memset(one[:], 1.0)
            nc.vector.tensor_copy(ones_sb[:], one[:])
            nc.vector.tensor_copy(p_img[:], _r0(zer, p_img))
            nc.vector.tensor_copy(v2_img[:], _r0(zer, v2_img))
            nc.vector.tensor_copy(f_img[:], _r0(zer, f_img))

            # valT[kpos, d] = sum_c v[c, kpos] * w_value[d, c]  (plain fp32:
            # N=128 gets no fp32r speedup and fp32 avoids rounding v/wv)
            for j in range(KT // 4):
                ps = psmm.tile([128, 512], F32, tag="mm", name="mm")
                for i in range(4):
                    kt = j * 4 + i
                    nc.tensor.matmul(
                        ps[:, i * 128 : (i + 1) * 128],
                        v_sb[:, kt * 128 : (kt + 1) * 128],
                        wv_sb[:],
                        start=True,
                        stop=True,
                    )
                nc.any.tensor_copy(valT[j][:], ps[:])

            # ---- stage emitters --------------------------------------
            def attn_qb(b):
                """scores -> exp -> pv/cs accumulate -> normalized p_img rows."""
                ps_pv = pspv.tile([128, QB], F32, tag="pv", name="pv")
                ps_cs = pscs.tile([128, QB], F32, tag="cs", name="cs")
                for kt in range(KT):
                    ps_s = psmm.tile([128, 512], F32, tag="mm", name="mm")[:, :QB]
                    for ct in range(CT):
                        nc.tensor.matmul(
                            ps_s,
                            k_t[kt][:, ct, :],
                            q_t[b][:, ct, :],
                            start=(ct == 0),
                            stop=(ct == CT - 1),
                        )
                    e_t = work.tile([128, QB], MMDT, tag="exp", name="exp")
                    nc.scalar.activation(e_t[:], ps_s, AF.Exp, scale=SCALE)
                    nc.tensor.matmul(
                        ps_pv[:],
                        valT[kt // 4][:, (kt % 4) * 128 : (kt % 4 + 1) * 128],
                        e_t[:],
                        start=(kt == 0),
                        stop=(kt == KT - 1),
                    )
                    nc.tensor.matmul(
                        ps_cs[:],
                        ones_sb[:],
                        e_t[:],
                        start=(kt == 0),
                        stop=(kt == KT - 1),
                    )
                rcp = work.tile([128, QB], F32, tag="rcp", name="rcp")
                nc.vector.reciprocal(rcp[:], ps_cs[:])
                nc.vector.tensor_mul(
                    p_img[:, 1 + b * RB : 1 + (b + 1) * RB, 1 : 1 + W],
                    _r3(ps_pv[:]),
                    _r3(rcp[:]),
                )

            def conv9(img, wgt_sb, dil, b):
                """3x3 conv (pad == dil) for output rows b*RB..b*RB+RB-1."""
                ps = pscv.tile([128, QB], F32, tag="cv", name="cv")
                for t in range(9):
                    ky, kx = divmod(t, 3)
                    rhs = img[
                        :,
                        b * RB + dil * ky : b * RB + dil * ky + RB,
                        dil * kx : dil * kx + W,
                    ]
                    nc.tensor.matmul(
                        ps[:],
                        wgt_sb[:, t, :],
                        rhs,
                        start=(t == 0),
                        stop=(t == 8),
                    )
                return ps

            def conv1_blk(b):
                # v2 = v + lrelu(conv3x3(p))
                ps = conv9(p_img, wo_sb, 1, b)
                lr = work.tile([128, QB], F32, tag="lr", name="lr")
                nc.vector.tensor_scalar_mul(lr[:], ps[:], LRELU_SLOPE)
                nc.vector.tensor_max(lr[:], lr[:], ps[:])
                nc.vector.tensor_add(
                    v2_img[:, 2 + b * RB : 2 + (b + 1) * RB, 2 : 2 + W],
                    _r3(lr[:]),
                    vl_sb[:, b * RB : (b + 1) * RB, :],
                )

            def conv2_blk(b):
                # f = lrelu(dconv3x3_d2(v2))
                ps = conv9(v2_img, wf1_sb, 2, b)
                lr = work.tile([128, QB], F32, tag="lr", name="lr")
                nc.vector.tensor_scalar_mul(lr[:], ps[:], LRELU_SLOPE)
                nc.vector.tensor_max(
                    f_img[:, 1 + b * RB : 1 + (b + 1) * RB, 1 : 1 + W],
                    _r3(lr[:]),
                    _r3(ps[:]),
                )

            def conv3_blk(b):
                # out = v2 + lrelu(conv3x3(f))
                ps = conv9(f_img, wf2_sb, 1, b)
                lr = work.tile([128, QB], F32, tag="lr", name="lr")
                nc.vector.tensor_scalar_mul(lr[:], ps[:], LRELU_SLOPE)
                nc.vector.tensor_max(lr[:], lr[:], ps[:])
                nc.vector.tensor_add(
                    out_sb[:, b * RB : (b + 1) * RB, :],
                    _r3(lr[:]),
                    _f32(v2_img[:, 2 + b * RB : 2 + (b + 1) * RB, 2 : 2 + W]),
                )
                nc.sync.dma_start(
                    out_d[:, b * RB : (b + 1) * RB, :],
                    out_sb[:, b * RB : (b + 1) * RB, :],
                )

            # ---- interleaved schedule: conv block b_i only needs image
            # rows from earlier-emitted producers, so the PE can flow from
            # attention into convs without a phase barrier.
            attn_qb(0)
            attn_qb(1)
            conv1_blk(0)
            attn_qb(2)
            conv1_blk(1)
            conv2_blk(0)
            attn_qb(3)
            conv1_blk(2)
            conv2_blk(1)
            conv3_blk(0)
            conv1_blk(3)
            conv2_blk(2)
            conv3_blk(1)
            conv2_blk(3)
            conv3_blk(2)
            conv3_blk(3)

    nc.finalize()
    return nc


def _r0(zer, img):
    """Slice of the zeros staging tile matching img's free size."""
    n = img.shape[1] * img.shape[2]
    return zer[:, :n].rearrange("p (r w) -> p r w", w=img.shape[2])


_NC_CACHE = []


def _get_nc():
    if not _NC_CACHE:
        _NC_CACHE.append(build_nc())
    return _NC_CACHE[0]


def _prep_core_inputs(k, q, v, wv_t, wo_t, wf1_t, wf2_t, n, r0):
    kn = k[n].reshape(CT, 128, KT, 128).transpose(1, 2, 0, 3)  # [128, kt, ct, 128]
    qn = (
        q[n]
        .reshape(CT, 128, H, W)[:, :, r0 : r0 + RQ, :]
        .reshape(CT, 128, NB, QB)
        .transpose(1, 2, 0, 3)
    )  # [128, qb, ct, QB]
    vn = v[n].reshape(D, HW)
    vl = v[n][:, r0 : r0 + RQ, :]
    return {
        "kk": np.ascontiguousarray(kn),
        "qq": np.ascontiguousarray(qn),
        "vv": np.ascontiguousarray(vn),
        "vloc": np.ascontiguousarray(vl),
        "wv": wv_t,
        "wo": wo_t,
        "wf1": wf1_t,
        "wf2": wf2_t,
    }


def kernel(k, q, v, w_value, w_out, w_ff1, w_ff2, _trace=False, _trace_kwargs=None):
    k = np.ascontiguousarray(np.asarray(k, dtype=np.float32))
    q = np.ascontiguousarray(np.asarray(q, dtype=np.float32))
    v = np.ascontiguousarray(np.asarray(v, dtype=np.float32))
    w_value = np.asarray(w_value, dtype=np.float32)
    w_out = np.asarray(w_out, dtype=np.float32)
    w_ff1 = np.asarray(w_ff1, dtype=np.float32)
    w_ff2 = np.asarray(w_ff2, dtype=np.float32)

    # per-tap transposed weights: [c_in, tap, c_out]
    wv_t = np.ascontiguousarray(w_value[:, :, 0, 0].T)
    wo_t = np.ascontiguousarray(w_out.transpose(1, 2, 3, 0).reshape(D, 9, D))
    wf1_t = np.ascontiguousarray(w_ff1.transpose(1, 2, 3, 0).reshape(D, 9, D))
    wf2_t = np.ascontiguousarray(w_ff2.transpose(1, 2, 3, 0).reshape(D, 9, D))

    in_maps = []
    for core in range(8):
        n, half = divmod(core, 2)
        r0 = 0 if half == 0 else H - RQ  # 0 or 20
        in_maps.append(_prep_core_inputs(k, q, v, wv_t, wo_t, wf1_t, wf2_t, n, r0))

    nc = _get_nc()
    kwargs = {}
    if _trace:
        kwargs = {"trace": True, **(_trace_kwargs or {})}
    res = run_bass_kernel_spmd(nc, in_maps, core_ids=list(range(8)), **kwargs)

    out = np.empty((N_BATCH, D, H, W), dtype=np.float32)
    for core in range(8):
        n, half = divmod(core, 2)
        local = res.results[core]["out"]  # [128, RQ, W]
        if half == 0:
            out[n, :, 0:24, :] = local[:, 0:24, :]
        else:
            out[n, :, 24:48, :] = local[:, RQ - 24 :, :]
    if _trace:
        return out, res
    return out


# revision 7
# speedup vs baseline: 1.1502x; 1.1502x over previous
"""Bass/Tile Trainium2 kernel for nn_CrossAttention (8 NeuronCores).

Sharding: data-parallel over batch N=4, and each sample's query positions
are split across 2 cores (rows 0..23 / 24..47 of the 48x64 image, plus a
4-row halo so the conv stack after attention needs no communication).

Per-core pipeline (uniform across cores; all asymmetry handled on host):
  valT = (w_value @ v)^T                       [kpos, d] tiles
  scores[k, q] = k^T q                         PE, fp32r
  e = exp(scores / sqrt(384))                  ACT, PSUM->SBUF (fp32r out)
  pv[d, q]  = val @ e                          PE accumulation over k-tiles
  cs[q]     = ones^T e  (broadcast to 128p)    PE accumulation over k-tiles
  p = pv / cs                                  DVE (reciprocal + mul)
  v2 = v + lrelu(conv3x3(p))                   PE + DVE
  f  = lrelu(dconv3x3_d2(v2))                  PE + DVE
  out = v2 + lrelu(conv3x3(f))                 PE + DVE

fp32r note: the PE runs fp32r (full-rate single-pass fp32) only when every
matmul operand was *produced* as fp32r by a compute instruction, so DMA'd
tensors are bounced through a rounding copy and on-chip producers (ACT exp,
DVE image writes) emit fp32r directly.
"""

import math

import numpy as np

import concourse.bass as bass
import concourse.mybir as mybir
import concourse.tile as tile
from concourse import bacc
from concourse.bass_utils import run_bass_kernel_spmd

F32 = mybir.dt.float32
F32R = mybir.dt.float32r
AF = mybir.ActivationFunctionType

# Problem constants (hardcoded per the contract)
N_BATCH = 4
C = 384            # kq channels
D = 128            # hidden
H, W = 48, 64
HW = H * W         # 3072
CT = C // 128      # 3 contraction tiles
KT = HW // 128     # 24 key-position tiles
RQ = 28            # local rows per core (24 output + 4 halo)
Q = RQ * W         # 1792 query positions per core
RB = 7             # image rows per block
NB = RQ // RB      # 4 blocks
QB = RB * W        # 448 block free size
SCALE = 1.0 / math.sqrt(C)
LRELU_SLOPE = 0.2

USE_F32R = True    # run the big matmuls in fp32r (full-rate fp32)

MMDT = F32R if USE_F32R else F32


def _r3(ap):
    """[128, QB] -> [128, RB, W] view."""
    return ap.rearrange("p (r w) -> p r w", w=W)


def _f32(ap):
    """Read an fp32r tile as plain fp32 (same bits) for DVE/ACT consumers."""
    return ap.bitcast(F32) if ap.dtype == F32R else ap


def build_nc():
    nc = bacc.Bacc(None, target_bir_lowering=False)

    kk = nc.dram_tensor("kk", [128, KT, CT, 128], MMDT, kind="ExternalInput")
    qq = nc.dram_tensor("qq", [128, NB, CT, QB], MMDT, kind="ExternalInput")
    vv = nc.dram_tensor("vv", [128, HW], F32, kind="ExternalInput")
    vloc = nc.dram_tensor("vloc", [128, RQ, W], F32, kind="ExternalInput")
    wv = nc.dram_tensor("wv", [128, 128], F32, kind="ExternalInput")
    wo = nc.dram_tensor("wo", [128, 9, 128], MMDT, kind="ExternalInput")
    wf1 = nc.dram_tensor("wf1", [128, 9, 128], MMDT, kind="ExternalInput")
    wf2 = nc.dram_tensor("wf2", [128, 9, 128], MMDT, kind="ExternalInput")
    out_d = nc.dram_tensor("out", [128, RQ, W], F32, kind="ExternalOutput")

    with tile.TileContext(nc) as tc:
        with (
            tc.tile_pool(name="io", bufs=1) as io,
            tc.tile_pool(name="work", bufs=4) as work,
            tc.tile_pool(name="psmm", bufs=4, space="PSUM") as psmm,
            tc.tile_pool(name="pspv", bufs=1, space="PSUM") as pspv,
            tc.tile_pool(name="pscs", bufs=1, space="PSUM") as pscs,
            tc.tile_pool(name="pscv", bufs=2, space="PSUM") as pscv,
        ):
            # persistent SBUF tensors (matmul operands in MMDT)
            k_t = [io.tile([128, CT, 128], MMDT, tag=f"k{kt}", name=f"k{kt}") for kt in range(KT)]
            q_t = [io.tile([128, CT, QB], MMDT, tag=f"q{b}", name=f"q{b}") for b in range(NB)]
            v_sb = io.tile([128, HW], F32, tag="v", name="v")
            vl_sb = io.tile([128, RQ, W], F32, tag="vl", name="vl")
            wv_sb = io.tile([128, 128], F32, tag="wv", name="wv")
            wo_sb = io.tile([128, 9, 128], MMDT, tag="wo", name="wo")
            wf1_sb = io.tile([128, 9, 128], MMDT, tag="wf1", name="wf1")
            wf2_sb = io.tile([128, 9, 128], MMDT, tag="wf2", name="wf2")
            ones_sb = io.tile([128, 128], MMDT, tag="ones", name="ones")
            valT = [io.tile([128, 512], MMDT, tag=f"vt{j}", name=f"vt{j}") for j in range(KT // 4)]
            p_img = io.tile([128, RQ + 2, W + 2], MMDT, tag="p_img", name="p_img")
            v2_img = io.tile([128, RQ + 4, W + 4], MMDT, tag="v2_img", name="v2_img")
            f_img = io.tile([128, RQ + 2, W + 2], MMDT, tag="f_img", name="f_img")
            out_sb = io.tile([128, RQ, W], F32, tag="out_sb", name="out_sb")

            # input DMAs; v + wv first (they feed valT, the first PE work),
            # then q block 0 + k tiles so scores can start ASAP
            nc.sync.dma_start(v_sb[:], vv[:])
            nc.sync.dma_start(wv_sb[:], wv[:])
            nc.sync.dma_start(q_t[0][:], qq[:, 0, :, :])
            for kt in range(KT):
                nc.sync.dma_start(k_t[kt][:], kk[:, kt, :, :])
            for b in range(1, NB):
                nc.sync.dma_start(q_t[b][:], qq[:, b, :, :])
            nc.sync.dma_start(vl_sb[:], vloc[:])
            nc.sync.dma_start(wo_sb[:], wo[:])
            nc.sync.dma_start(wf1_sb[:], wf1[:])
            nc.sync.dma_start(wf2_sb[:], wf2[:])

            # constants / zero borders, produced by compute ops so the
            # fp32r verifier sees rounded producers
            zer = io.tile([128, (RQ + 4) * (W + 4)], F32, tag="zer", name="zer")
            nc.vector.memset(zer[:], 0.0)
            one = io.tile([128, 128], F32, tag="one", name="one")
            nc.vector.memset(one[:], 1.0)
            nc.vector.tensor_copy(ones_sb[:], one[:])
            nc.vector.tensor_copy(p_img[:], _r0(zer, p_img))
            nc.vector.tensor_copy(v2_img[:], _r0(zer, v2_img))
            nc.vector.tensor_copy(f_img[:], _r0(zer, f_img))

            # valT[kpos, d] = sum_c v[c, kpos] * w_value[d, c]  (plain fp32:
            # N=128 gets no fp32r speedup and fp32 avoids rounding v/wv)
            for j in range(KT // 4):
                ps = psmm.tile([128, 512], F32, tag="mm", name="mm")
                for i in range(4):
                    kt = j * 4 + i
                    nc.tensor.matmul(
                        ps[:, i * 128 : (i + 1) * 128],
                        v_sb[:, kt * 128 : (kt + 1) * 128],
                        wv_sb[:],
                        start=True,
                        stop=True,
                    )
                nc.any.tensor_copy(valT[j][:], ps[:])

            # ---- stage emitters --------------------------------------
            def attn_qb(b):
                """scores -> exp -> pv/cs accumulate -> normalized p_img rows."""
                ps_pv = pspv.tile([128, QB], F32, tag="pv", name="pv")
                ps_cs = pscs.tile([128, QB], F32, tag="cs", name="cs")
                for kt in range(KT):
                    ps_s = psmm.tile([128, 512], F32, tag="mm", name="mm")[:, :QB]
                    for ct in range(CT):
                        nc.tensor.matmul(
                            ps_s,
                            k_t[kt][:, ct, :],
                            q_t[b][:, ct, :],
                            start=(ct == 0),
                            stop=(ct == CT - 1),
                        )
                    e_t = work.tile([128, QB], MMDT, tag="exp", name="exp")
                    nc.scalar.activation(e_t[:], ps_s, AF.Exp, scale=SCALE)
                    nc.tensor.matmul(
                        ps_pv[:],
                        valT[kt // 4][:, (kt % 4) * 128 : (kt % 4 + 1) * 128],
                        e_t[:],
                        start=(kt == 0),
                        stop=(kt == KT - 1),
                    )
                    nc.tensor.matmul(
                        ps_cs[:],
                        ones_sb[:],
                        e_t[:],
                        start=(kt == 0),
                        stop=(kt == KT - 1),
                    )
                pv_sb = work.tile([128, QB], F32, tag="pvsb", name="pvsb")
                nc.any.tensor_copy(pv_sb[:], ps_pv[:])
                cs_sb = work.tile([128, QB], F32, tag="cssb", name="cssb")
                nc.any.tensor_copy(cs_sb[:], ps_cs[:])
                rcp = work.tile([128, QB], F32, tag="rcp", name="rcp")
                nc.vector.reciprocal(rcp[:], cs_sb[:])
                nc.vector.tensor_mul(
                    p_img[:, 1 + b * RB : 1 + (b + 1) * RB, 1 : 1 + W],
                    _r3(pv_sb[:]),
                    _r3(rcp[:]),
                )

            def conv9(img, wgt_sb, dil, b):
                """3x3 conv (pad == dil) for output rows b*RB..b*RB+RB-1."""
                ps = pscv.tile([128, QB], F32, tag="cv", name="cv")
                for t in range(9):
                    ky, kx = divmod(t, 3)
                    rhs = img[
                        :,
                        b * RB + dil * ky : b * RB + dil * ky + RB,
                        dil * kx : dil * kx + W,
                    ]
                    nc.tensor.matmul(
                        ps[:],
                        wgt_sb[:, t, :],
                        rhs,
                        start=(t == 0),
                        stop=(t == 8),
                    )
                return ps

            def conv1_blk(b):
                # v2 = v + lrelu(conv3x3(p))
                ps = conv9(p_img, wo_sb, 1, b)
                lr = work.tile([128, QB], F32, tag="lr", name="lr")
                nc.vector.tensor_scalar_mul(lr[:], ps[:], LRELU_SLOPE)
                nc.vector.tensor_max(lr[:], lr[:], ps[:])
                nc.vector.tensor_add(
                    v2_img[:, 2 + b * RB : 2 + (b + 1) * RB, 2 : 2 + W],
                    _r3(lr[:]),
                    vl_sb[:, b * RB : (b + 1) * RB, :],
                )

            def conv2_blk(b):
                # f = lrelu(dconv3x3_d2(v2))
                ps = conv9(v2_img, wf1_sb, 2, b)
                lr = work.tile([128, QB], F32, tag="lr", name="lr")
                nc.vector.tensor_scalar_mul(lr[:], ps[:], LRELU_SLOPE)
                nc.vector.tensor_max(
                    f_img[:, 1 + b * RB : 1 + (b + 1) * RB, 1 : 1 + W],
                    _r3(lr[:]),
                    _r3(ps[:]),
                )

            def conv3_blk(b):
                # out = v2 + lrelu(conv3x3(f))
                ps = conv9(f_img, wf2_sb, 1, b)
                lr = work.tile([128, QB], F32, tag="lr", name="lr")
                nc.vector.tensor_scalar_mul(lr[:], ps[:], LRELU_SLOPE)
                nc.vector.tensor_max(lr[:], lr[:], ps[:])
                nc.vector.tensor_add(
                    out_sb[:, b * RB : (b + 1) * RB, :],
                    _r3(lr[:]),
                    _f32(v2_img[:, 2 + b * RB : 2 + (b + 1) * RB, 2 : 2 + W]),
                )
                nc.sync.dma_start(
                    out_d[:, b * RB : (b + 1) * RB, :],
                    out_sb[:, b * RB : (b + 1) * RB, :],
                )

            # ---- interleaved schedule: conv block b_i only needs image
            # rows from earlier-emitted producers, so the PE can flow from
            # attention into convs without a phase barrier.
            attn_qb(0)
            attn_qb(1)
            conv1_blk(0)
            attn_qb(2)
            conv1_blk(1)
            conv2_blk(0)
            attn_qb(3)
            conv1_blk(2)
            conv2_blk(1)
            conv3_blk(0)
            conv1_blk(3)
            conv2_blk(2)
            conv3_blk(1)
            conv2_blk(3)
            conv3_blk(2)
            conv3_blk(3)

    nc.finalize()
    return nc


def _r0(zer, img):
    """Slice of the zeros staging tile matching img's free size."""
    n = img.shape[1] * img.shape[2]
    return zer[:, :n].rearrange("p (r w) -> p r w", w=img.shape[2])


_NC_CACHE = []


def _get_nc():
    if not _NC_CACHE:
        _NC_CACHE.append(build_nc())
    return _NC_CACHE[0]


def _prep_core_inputs(k, q, v, wv_t, wo_t, wf1_t, wf2_t, n, r0):
    kn = k[n].reshape(CT, 128, KT, 128).transpose(1, 2, 0, 3)  # [128, kt, ct, 128]
    qn = (
        q[n]
        .reshape(CT, 128, H, W)[:, :, r0 : r0 + RQ, :]
        .reshape(CT, 128, NB, QB)
        .transpose(1, 2, 0, 3)
    )  # [128, qb, ct, QB]
    vn = v[n].reshape(D, HW)
    vl = v[n][:, r0 : r0 + RQ, :]
    return {
        "kk": np.ascontiguousarray(kn),
        "qq": np.ascontiguousarray(qn),
        "vv": np.ascontiguousarray(vn),
        "vloc": np.ascontiguousarray(vl),
        "wv": wv_t,
        "wo": wo_t,
        "wf1": wf1_t,
        "wf2": wf2_t,
    }


def kernel(k, q, v, w_value, w_out, w_ff1, w_ff2, _trace=False, _trace_kwargs=None):
    k = np.ascontiguousarray(np.asarray(k, dtype=np.float32))
    q = np.ascontiguousarray(np.asarray(q, dtype=np.float32))
    v = np.ascontiguousarray(np.asarray(v, dtype=np.float32))
    w_value = np.asarray(w_value, dtype=np.float32)
    w_out = np.asarray(w_out, dtype=np.float32)
    w_ff1 = np.asarray(w_ff1, dtype=np.float32)
    w_ff2 = np.asarray(w_ff2, dtype=np.float32)

    # per-tap transposed weights: [c_in, tap, c_out]
    wv_t = np.ascontiguousarray(w_value[:, :, 0, 0].T)
    wo_t = np.ascontiguousarray(w_out.transpose(1, 2, 3, 0).reshape(D, 9, D))
    wf1_t = np.ascontiguousarray(w_ff1.transpose(1, 2, 3, 0).reshape(D, 9, D))
    wf2_t = np.ascontiguousarray(w_ff2.transpose(1, 2, 3, 0).reshape(D, 9, D))

    in_maps = []
    for core in range(8):
        n, half = divmod(core, 2)
        r0 = 0 if half == 0 else H - RQ  # 0 or 20
        in_maps.append(_prep_core_inputs(k, q, v, wv_t, wo_t, wf1_t, wf2_t, n, r0))

    nc = _get_nc()
    kwargs = {}
    if _trace:
        kwargs = {"trace": True, **(_trace_kwargs or {})}
    res = run_bass_kernel_spmd(nc, in_maps, core_ids=list(range(8)), **kwargs)

    out = np.empty((N_BATCH, D, H, W), dtype=np.float32)
    for core in range(8):
        n, half = divmod(core, 2)
        local = res.results[core]["out"]  # [128, RQ, W]
        if half == 0:
            out[n, :, 0:24, :] = local[:, 0:24, :]
        else:
            out[n, :, 24:48, :] = local[:, RQ - 24 :, :]
    if _trace:
        return out, res
    return out


# revision 9
# speedup vs baseline: 1.1826x; 1.0281x over previous
"""Bass/Tile Trainium2 kernel for nn_CrossAttention (8 NeuronCores).

Sharding: data-parallel over batch N=4, and each sample's query positions
are split across 2 cores (rows 0..23 / 24..47 of the 48x64 image, plus a
4-row halo so the conv stack after attention needs no communication).

Per-core pipeline (uniform across cores; all asymmetry handled on host):
  valT = (w_value @ v)^T                       [kpos, d] tiles
  scores[k, q] = k^T q                         PE, fp32r
  e = exp(scores / sqrt(384))                  ACT, PSUM->SBUF (fp32r out)
  pv[d, q]  = val @ e                          PE accumulation over k-tiles
  cs[q]     = ones^T e  (broadcast to 128p)    PE accumulation over k-tiles
  p = pv / cs                                  DVE (reciprocal + mul)
  v2 = v + lrelu(conv3x3(p))                   PE + DVE
  f  = lrelu(dconv3x3_d2(v2))                  PE + DVE
  out = v2 + lrelu(conv3x3(f))                 PE + DVE

fp32r note: the PE runs fp32r (full-rate single-pass fp32) only when every
matmul operand was *produced* as fp32r by a compute instruction, so DMA'd
tensors are bounced through a rounding copy and on-chip producers (ACT exp,
DVE image writes) emit fp32r directly.
"""

import math

import numpy as np

import concourse.bass as bass
import concourse.mybir as mybir
import concourse.tile as tile
from concourse import bacc
from concourse.bass_utils import run_bass_kernel_spmd

F32 = mybir.dt.float32
F32R = mybir.dt.float32r
AF = mybir.ActivationFunctionType
ALU = mybir.AluOpType

# Problem constants (hardcoded per the contract)
N_BATCH = 4
C = 384            # kq channels
D = 128            # hidden
H, W = 48, 64
HW = H * W         # 3072
CT = C // 128      # 3 contraction tiles
KT = HW // 128     # 24 key-position tiles
RQ = 28            # local rows per core (24 output + 4 halo)
Q = RQ * W         # 1792 query positions per core
RB = 7             # image rows per block
NB = RQ // RB      # 4 blocks
QB = RB * W        # 448 block free size
SCALE = 1.0 / math.sqrt(C)
LRELU_SLOPE = 0.2

USE_F32R = True    # run the big matmuls in fp32r (full-rate fp32)

MMDT = F32R if USE_F32R else F32


def _r3(ap):
    """[128, QB] -> [128, RB, W] view."""
    return ap.rearrange("p (r w) -> p r w", w=W)


def _f32(ap):
    """Read an fp32r tile as plain fp32 (same bits) for DVE/ACT consumers."""
    return ap.bitcast(F32) if ap.dtype == F32R else ap


def build_nc():
    nc = bacc.Bacc(None, target_bir_lowering=False)

    kk = nc.dram_tensor("kk", [128, KT, CT, 128], MMDT, kind="ExternalInput")
    qq = nc.dram_tensor("qq", [128, NB, CT, QB], MMDT, kind="ExternalInput")
    vv = nc.dram_tensor("vv", [128, HW], F32, kind="ExternalInput")
    vloc = nc.dram_tensor("vloc", [128, RQ, W], F32, kind="ExternalInput")
    wv = nc.dram_tensor("wv", [128, 128], F32, kind="ExternalInput")
    wo = nc.dram_tensor("wo", [128, 9, 128], MMDT, kind="ExternalInput")
    wf1 = nc.dram_tensor("wf1", [128, 9, 128], MMDT, kind="ExternalInput")
    wf2 = nc.dram_tensor("wf2", [128, 9, 128], MMDT, kind="ExternalInput")
    out_d = nc.dram_tensor("out", [128, RQ, W], F32, kind="ExternalOutput")

    with tile.TileContext(nc) as tc:
        with (
            tc.tile_pool(name="io", bufs=1) as io,
            tc.tile_pool(name="work", bufs=4) as work,
            tc.tile_pool(name="psmm", bufs=4, space="PSUM") as psmm,
            tc.tile_pool(name="pspv", bufs=2, space="PSUM") as pspv,
            tc.tile_pool(name="pscs", bufs=2, space="PSUM") as pscs,
        ):
            # persistent SBUF tensors (matmul operands in MMDT)
            k_t = [io.tile([128, CT, 128], MMDT, tag=f"k{kt}", name=f"k{kt}") for kt in range(KT)]
            q_t = [io.tile([128, CT, QB], MMDT, tag=f"q{b}", name=f"q{b}") for b in range(NB)]
            v_sb = io.tile([128, HW], F32, tag="v", name="v")
            vl_sb = io.tile([128, RQ, W], F32, tag="vl", name="vl")
            wv_sb = io.tile([128, 128], F32, tag="wv", name="wv")
            wo_sb = io.tile([128, 9, 128], MMDT, tag="wo", name="wo")
            wf1_sb = io.tile([128, 9, 128], MMDT, tag="wf1", name="wf1")
            wf2_sb = io.tile([128, 9, 128], MMDT, tag="wf2", name="wf2")
            ones_sb = io.tile([128, 128], MMDT, tag="ones", name="ones")
            valT = [io.tile([128, 512], MMDT, tag=f"vt{j}", name=f"vt{j}") for j in range(KT // 4)]
            p_img = io.tile([128, RQ + 2, W + 2], MMDT, tag="p_img", name="p_img")
            v2_img = io.tile([128, RQ + 4, W + 4], MMDT, tag="v2_img", name="v2_img")
            f_img = io.tile([128, RQ + 2, W + 2], MMDT, tag="f_img", name="f_img")
            out_sb = io.tile([128, RQ, W], F32, tag="out_sb", name="out_sb")

            # input DMAs; v + wv first (they feed valT, the first PE work),
            # then q block 0 + k tiles so scores can start ASAP
            nc.sync.dma_start(wv_sb[:], wv[:])
            for j in range(KT // 4):
                nc.sync.dma_start(
                    v_sb[:, j * 512 : (j + 1) * 512], vv[:, j * 512 : (j + 1) * 512]
                )
            nc.sync.dma_start(q_t[0][:], qq[:, 0, :, :])
            for kt in range(KT):
                nc.sync.dma_start(k_t[kt][:], kk[:, kt, :, :])
            for b in range(1, NB):
                nc.sync.dma_start(q_t[b][:], qq[:, b, :, :])
            nc.sync.dma_start(vl_sb[:], vloc[:])
            nc.sync.dma_start(wo_sb[:], wo[:])
            nc.sync.dma_start(wf1_sb[:], wf1[:])
            nc.sync.dma_start(wf2_sb[:], wf2[:])

            # constants / zero borders, produced by compute ops so the
            # fp32r verifier sees rounded producers
            zer = io.tile([128, (RQ + 4) * (W + 4)], F32, tag="zer", name="zer")
            nc.vector.memset(zer[:], 0.0)
            one = io.tile([128, 128], F32, tag="one", name="one")
            nc.vector.memset(one[:], 1.0)
            nc.vector.tensor_copy(ones_sb[:], one[:])
            nc.vector.tensor_copy(p_img[:], _r0(zer, p_img))
            nc.vector.tensor_copy(v2_img[:], _r0(zer, v2_img))
            nc.vector.tensor_copy(f_img[:], _r0(zer, f_img))

            # valT[kpos, d] = sum_c v[c, kpos] * w_value[d, c]  (plain fp32:
            # N=128 gets no fp32r speedup and fp32 avoids rounding v/wv)
            for j in range(KT // 4):
                ps = psmm.tile([128, 512], F32, tag="mm", name="mm")
                for i in range(4):
                    kt = j * 4 + i
                    nc.tensor.matmul(
                        ps[:, i * 128 : (i + 1) * 128],
                        v_sb[:, kt * 128 : (kt + 1) * 128],
                        wv_sb[:],
                        start=True,
                        stop=True,
                    )
                nc.any.tensor_copy(valT[j][:], ps[:])

            # ---- stage emitters --------------------------------------
            def attn_qb(b):
                """scores -> exp -> pv/cs accumulate -> normalized p_img rows."""
                ps_pv = pspv.tile([128, QB], F32, tag="pv", name="pv")
                ps_cs = pscs.tile([128, QB], F32, tag="cs", name="cs")
                for kt in range(KT):
                    ps_s = psmm.tile([128, 512], F32, tag="mm", name="mm")[:, :QB]
                    for ct in range(CT):
                        nc.tensor.matmul(
                            ps_s,
                            k_t[kt][:, ct, :],
                            q_t[b][:, ct, :],
                            start=(ct == 0),
                            stop=(ct == CT - 1),
                        )
                    e_t = work.tile([128, QB], MMDT, tag="exp", name="exp")
                    nc.scalar.activation(e_t[:], ps_s, AF.Exp, scale=SCALE)
                    nc.tensor.matmul(
                        ps_pv[:],
                        valT[kt // 4][:, (kt % 4) * 128 : (kt % 4 + 1) * 128],
                        e_t[:],
                        start=(kt == 0),
                        stop=(kt == KT - 1),
                    )
                    nc.tensor.matmul(
                        ps_cs[:],
                        ones_sb[:],
                        e_t[:],
                        start=(kt == 0),
                        stop=(kt == KT - 1),
                    )
                pv_sb = work.tile([128, QB], F32, tag="pvsb", name="pvsb")
                nc.any.tensor_copy(pv_sb[:], ps_pv[:])
                cs_sb = work.tile([128, QB], F32, tag="cssb", name="cssb")
                nc.any.tensor_copy(cs_sb[:], ps_cs[:])
                rcp = work.tile([128, QB], F32, tag="rcp", name="rcp")
                nc.vector.reciprocal(rcp[:], cs_sb[:])
                nc.vector.tensor_mul(
                    p_img[:, 1 + b * RB : 1 + (b + 1) * RB, 1 : 1 + W],
                    _r3(pv_sb[:]),
                    _r3(rcp[:]),
                )

            def conv9(img, wgt_sb, dil, b):
                """3x3 conv (pad == dil) for output rows b*RB..b*RB+RB-1."""
                ps = psmm.tile([128, 512], F32, tag="mm", name="mm")[:, :QB]
                for t in range(9):
                    ky, kx = divmod(t, 3)
                    rhs = img[
                        :,
                        b * RB + dil * ky : b * RB + dil * ky + RB,
                        dil * kx : dil * kx + W,
                    ]
                    nc.tensor.matmul(
                        ps[:],
                        wgt_sb[:, t, :],
                        rhs,
                        start=(t == 0),
                        stop=(t == 8),
                    )
                return ps

            def conv1_blk(b):
                # v2 = v + lrelu(conv3x3(p))
                ps = conv9(p_img, wo_sb, 1, b)
                lr = work.tile([128, QB], F32, tag="lr", name="lr")
                nc.scalar.mul(lr[:], ps[:], LRELU_SLOPE)
                nc.vector.tensor_max(lr[:], lr[:], ps[:])
                nc.vector.tensor_add(
                    v2_img[:, 2 + b * RB : 2 + (b + 1) * RB, 2 : 2 + W],
                    _r3(lr[:]),
                    vl_sb[:, b * RB : (b + 1) * RB, :],
                )

            def conv2_blk(b):
                # f = lrelu(dconv3x3_d2(v2))
                ps = conv9(v2_img, wf1_sb, 2, b)
                lr = work.tile([128, QB], F32, tag="lr", name="lr")
                nc.scalar.mul(lr[:], ps[:], LRELU_SLOPE)
                nc.vector.tensor_max(
                    f_img[:, 1 + b * RB : 1 + (b + 1) * RB, 1 : 1 + W],
                    _r3(lr[:]),
                    _r3(ps[:]),
                )

            def conv3_blk(b):
                # out = v2 + lrelu(conv3x3(f))
                ps = conv9(f_img, wf2_sb, 1, b)
                lr = work.tile([128, QB], F32, tag="lr", name="lr")
                nc.scalar.mul(lr[:], ps[:], LRELU_SLOPE)
                nc.vector.tensor_max(lr[:], lr[:], ps[:])
                nc.vector.tensor_add(
                    out_sb[:, b * RB : (b + 1) * RB, :],
                    _r3(lr[:]),
                    _f32(v2_img[:, 2 + b * RB : 2 + (b + 1) * RB, 2 : 2 + W]),
                )
                nc.sync.dma_start(
                    out_d[:, b * RB : (b + 1) * RB, :],
                    out_sb[:, b * RB : (b + 1) * RB, :],
                )

            # ---- interleaved schedule: conv block b_i only needs image
            # rows from earlier-emitted producers, so the PE can flow from
            # attention into convs without a phase barrier.
            attn_qb(0)
            attn_qb(1)
            conv1_blk(0)
            attn_qb(2)
            conv1_blk(1)
            conv2_blk(0)
            attn_qb(3)
            conv1_blk(2)
            conv2_blk(1)
            conv3_blk(0)
            conv1_blk(3)
            conv2_blk(2)
            conv3_blk(1)
            conv2_blk(3)
            conv3_blk(2)
            conv3_blk(3)

    nc.finalize()
    return nc


def _r0(zer, img):
    """Slice of the zeros staging tile matching img's free size."""
    n = img.shape[1] * img.shape[2]
    return zer[:, :n].rearrange("p (r w) -> p r w", w=img.shape[2])


_NC_CACHE = []


def _get_nc():
    if not _NC_CACHE:
        _NC_CACHE.append(build_nc())
    return _NC_CACHE[0]


def _prep_core_inputs(k, q, v, wv_t, wo_t, wf1_t, wf2_t, n, r0):
    kn = k[n].reshape(CT, 128, KT, 128).transpose(1, 2, 0, 3)  # [128, kt, ct, 128]
    qn = (
        q[n]
        .reshape(CT, 128, H, W)[:, :, r0 : r0 + RQ, :]
        .reshape(CT, 128, NB, QB)
        .transpose(1, 2, 0, 3)
    )  # [128, qb, ct, QB]
    vn = v[n].reshape(D, HW)
    vl = v[n][:, r0 : r0 + RQ, :]
    return {
        "kk": np.ascontiguousarray(kn),
        "qq": np.ascontiguousarray(qn),
        "vv": np.ascontiguousarray(vn),
        "vloc": np.ascontiguousarray(vl),
        "wv": wv_t,
        "wo": wo_t,
        "wf1": wf1_t,
        "wf2": wf2_t,
    }


def kernel(k, q, v, w_value, w_out, w_ff1, w_ff2, _trace=False, _trace_kwargs=None):
    k = np.ascontiguousarray(np.asarray(k, dtype=np.float32))
    q = np.ascontiguousarray(np.asarray(q, dtype=np.float32))
    v = np.ascontiguousarray(np.asarray(v, dtype=np.float32))
    w_value = np.asarray(w_value, dtype=np.float32)
    w_out = np.asarray(w_out, dtype=np.float32)
    w_ff1 = np.asarray(w_ff1, dtype=np.float32)
    w_ff2 = np.asarray(w_ff2, dtype=np.float32)

    # per-tap transposed weights: [c_in, tap, c_out]
    wv_t = np.ascontiguousarray(w_value[:, :, 0, 0].T)
    wo_t = np.ascontiguousarray(w_out.transpose(1, 2, 3, 0).reshape(D, 9, D))
    wf1_t = np.ascontiguousarray(w_ff1.transpose(1, 2, 3, 0).reshape(D, 9, D))
    wf2_t = np.ascontiguousarray(w_ff2.transpose(1, 2, 3, 0).reshape(D, 9, D))

    in_maps = []
    for core in range(8):
        n, half = divmod(core, 2)
        r0 = 0 if half == 0 else H - RQ  # 0 or 20
        in_maps.append(_prep_core_inputs(k, q, v, wv_t, wo_t, wf1_t, wf2_t, n, r0))

    nc = _get_nc()
    kwargs = {}
    if _trace:
        kwargs = {"trace": True, **(_trace_kwargs or {})}
    res = run_bass_kernel_spmd(nc, in_maps, core_ids=list(range(8)), **kwargs)

    out = np.empty((N_BATCH, D, H, W), dtype=np.float32)
    for core in range(8):
        n, half = divmod(core, 2)
        local = res.results[core]["out"]  # [128, RQ, W]
        if half == 0:
            out[n, :, 0:24, :] = local[:, 0:24, :]
        else:
            out[n, :, 24:48, :] = local[:, RQ - 24 :, :]
    if _trace:
        return out, res
    return out


# revision 10
# speedup vs baseline: 1.2369x; 1.0459x over previous
"""Bass/Tile Trainium2 kernel for nn_CrossAttention (8 NeuronCores).

Sharding: data-parallel over batch N=4, and each sample's query positions
are split across 2 cores (rows 0..23 / 24..47 of the 48x64 image, plus a
4-row halo so the conv stack after attention needs no communication).

Per-core pipeline (uniform across cores; all asymmetry handled on host):
  valT = (w_value @ v)^T                       [kpos, d] tiles
  scores[k, q] = k^T q                         PE, fp32r
  e = exp(scores / sqrt(384))                  ACT, PSUM->SBUF (fp32r out)
  pv[d, q]  = val @ e                          PE accumulation over k-tiles
  cs[q]     = ones^T e  (broadcast to 128p)    PE accumulation over k-tiles
  p = pv / cs                                  DVE (reciprocal + mul)
  v2 = v + lrelu(conv3x3(p))                   PE + DVE
  f  = lrelu(dconv3x3_d2(v2))                  PE + DVE
  out = v2 + lrelu(conv3x3(f))                 PE + DVE

fp32r note: the PE runs fp32r (full-rate single-pass fp32) only when every
matmul operand was *produced* as fp32r by a compute instruction, so DMA'd
tensors are bounced through a rounding copy and on-chip producers (ACT exp,
DVE image writes) emit fp32r directly.
"""

import math

import numpy as np

import concourse.bass as bass
import concourse.mybir as mybir
import concourse.tile as tile
from concourse import bacc
from concourse.bass_utils import run_bass_kernel_spmd

F32 = mybir.dt.float32
F32R = mybir.dt.float32r
AF = mybir.ActivationFunctionType
ALU = mybir.AluOpType

# Problem constants (hardcoded per the contract)
N_BATCH = 4
C = 384            # kq channels
D = 128            # hidden
H, W = 48, 64
HW = H * W         # 3072
CT = C // 128      # 3 contraction tiles
KT = HW // 128     # 24 key-position tiles
RQ = 28            # local rows per core (24 output + 4 halo)
Q = RQ * W         # 1792 query positions per core
RB = 7             # image rows per block
NB = RQ // RB      # 4 blocks
QB = RB * W        # 448 block free size
SCALE = 1.0 / math.sqrt(C)
LRELU_SLOPE = 0.2

USE_F32R = True    # run the big matmuls in fp32r (full-rate fp32)

MMDT = F32R if USE_F32R else F32


def _r3(ap):
    """[128, QB] -> [128, RB, W] view."""
    return ap.rearrange("p (r w) -> p r w", w=W)


def _f32(ap):
    """Read an fp32r tile as plain fp32 (same bits) for DVE/ACT consumers."""
    return ap.bitcast(F32) if ap.dtype == F32R else ap


def build_nc():
    nc = bacc.Bacc(None, target_bir_lowering=False)

    kk = nc.dram_tensor("kk", [128, KT, CT, 128], MMDT, kind="ExternalInput")
    qq = nc.dram_tensor("qq", [128, NB, CT, QB], MMDT, kind="ExternalInput")
    vv = nc.dram_tensor("vv", [128, HW], F32, kind="ExternalInput")
    vloc = nc.dram_tensor("vloc", [128, RQ, W], F32, kind="ExternalInput")
    wv = nc.dram_tensor("wv", [128, 128], F32, kind="ExternalInput")
    wo = nc.dram_tensor("wo", [128, 9, 128], MMDT, kind="ExternalInput")
    wf1 = nc.dram_tensor("wf1", [128, 9, 128], MMDT, kind="ExternalInput")
    wf2 = nc.dram_tensor("wf2", [128, 9, 128], MMDT, kind="ExternalInput")
    out_d = nc.dram_tensor("out", [128, RQ, W], F32, kind="ExternalOutput")

    with tile.TileContext(nc) as tc:
        with (
            tc.tile_pool(name="io", bufs=1) as io,
            tc.tile_pool(name="work", bufs=4) as work,
            tc.tile_pool(name="psmm", bufs=4, space="PSUM") as psmm,
            tc.tile_pool(name="pspv", bufs=2, space="PSUM") as pspv,
            tc.tile_pool(name="pscs", bufs=2, space="PSUM") as pscs,
        ):
            # persistent SBUF tensors (matmul operands in MMDT)
            k_t = [io.tile([128, CT, 128], MMDT, tag=f"k{kt}", name=f"k{kt}") for kt in range(KT)]
            q_t = [io.tile([128, CT, QB], MMDT, tag=f"q{b}", name=f"q{b}") for b in range(NB)]
            v_sb = io.tile([128, HW], F32, tag="v", name="v")
            vl_sb = io.tile([128, RQ, W], F32, tag="vl", name="vl")
            wv_sb = io.tile([128, 128], F32, tag="wv", name="wv")
            wo_sb = io.tile([128, 9, 128], MMDT, tag="wo", name="wo")
            wf1_sb = io.tile([128, 9, 128], MMDT, tag="wf1", name="wf1")
            wf2_sb = io.tile([128, 9, 128], MMDT, tag="wf2", name="wf2")
            ones_sb = io.tile([128, 128], MMDT, tag="ones", name="ones")
            valT = [io.tile([128, 512], MMDT, tag=f"vt{j}", name=f"vt{j}") for j in range(KT // 4)]
            p_img = io.tile([128, RQ + 2, W + 2], MMDT, tag="p_img", name="p_img")
            v2_img = io.tile([128, RQ + 4, W + 4], MMDT, tag="v2_img", name="v2_img")
            f_img = io.tile([128, RQ + 2, W + 2], MMDT, tag="f_img", name="f_img")
            out_sb = io.tile([128, RQ, W], F32, tag="out_sb", name="out_sb")

            # input DMAs; v + wv first (they feed valT, the first PE work),
            # then q block 0 + k tiles so scores can start ASAP
            nc.sync.dma_start(wv_sb[:], wv[:])
            nc.sync.dma_start(v_sb[:, 0:512], vv[:, 0:512])
            nc.sync.dma_start(q_t[0][:], qq[:, 0, :, :])
            for kt in range(KT):
                nc.sync.dma_start(k_t[kt][:], kk[:, kt, :, :])
                if kt < KT // 4 - 1:
                    j = kt + 1
                    nc.sync.dma_start(
                        v_sb[:, j * 512 : (j + 1) * 512],
                        vv[:, j * 512 : (j + 1) * 512],
                    )
            for b in range(1, NB):
                nc.sync.dma_start(q_t[b][:], qq[:, b, :, :])
            nc.sync.dma_start(vl_sb[:], vloc[:])
            nc.sync.dma_start(wo_sb[:], wo[:])
            nc.sync.dma_start(wf1_sb[:], wf1[:])
            nc.sync.dma_start(wf2_sb[:], wf2[:])

            # constants / zero borders, produced by compute ops so the
            # fp32r verifier sees rounded producers
            zer = io.tile([128, (RQ + 4) * (W + 4)], F32, tag="zer", name="zer")
            nc.vector.memset(zer[:], 0.0)
            one = io.tile([128, 128], F32, tag="one", name="one")
            nc.vector.memset(one[:], 1.0)
            nc.vector.tensor_copy(ones_sb[:], one[:])
            nc.vector.tensor_copy(p_img[:], _r0(zer, p_img))
            nc.vector.tensor_copy(v2_img[:], _r0(zer, v2_img))
            nc.vector.tensor_copy(f_img[:], _r0(zer, f_img))

            # valT[kpos, d] = sum_c v[c, kpos] * w_value[d, c]  (plain fp32:
            # N=128 gets no fp32r speedup and fp32 avoids rounding v/wv)
            for j in range(KT // 4):
                ps = psmm.tile([128, 512], F32, tag="mm", name="mm")
                for i in range(4):
                    kt = j * 4 + i
                    nc.tensor.matmul(
                        ps[:, i * 128 : (i + 1) * 128],
                        v_sb[:, kt * 128 : (kt + 1) * 128],
                        wv_sb[:],
                        start=True,
                        stop=True,
                    )
                nc.any.tensor_copy(valT[j][:], ps[:])

            # ---- stage emitters --------------------------------------
            # software-pipelined attention: the pv/cs consumption of e_t(kt)
            # is emitted one k-tile late, so the PE never sits at the head of
            # its FIFO waiting for the exp of the tile it just produced.
            pend = []

            def _flush_pend():
                if not pend:
                    return
                b, kt, e_t, ps_pv, ps_cs = pend.pop()
                nc.tensor.matmul(
                    ps_pv[:],
                    valT[kt // 4][:, (kt % 4) * 128 : (kt % 4 + 1) * 128],
                    e_t[:],
                    start=(kt == 0),
                    stop=(kt == KT - 1),
                )
                nc.tensor.matmul(
                    ps_cs[:],
                    ones_sb[:],
                    e_t[:],
                    start=(kt == 0),
                    stop=(kt == KT - 1),
                )
                if kt == KT - 1:
                    pv_sb = work.tile([128, QB], F32, tag="pvsb", name="pvsb")
                    nc.any.tensor_copy(pv_sb[:], ps_pv[:])
                    cs_sb = work.tile([128, QB], F32, tag="cssb", name="cssb")
                    nc.any.tensor_copy(cs_sb[:], ps_cs[:])
                    rcp = work.tile([128, QB], F32, tag="rcp", name="rcp")
                    nc.vector.reciprocal(rcp[:], cs_sb[:])
                    nc.vector.tensor_mul(
                        p_img[:, 1 + b * RB : 1 + (b + 1) * RB, 1 : 1 + W],
                        _r3(pv_sb[:]),
                        _r3(rcp[:]),
                    )

            def attn_qb(b):
                """scores -> exp; pv/cs emission trails by one k-tile."""
                ps_pv = pspv.tile([128, QB], F32, tag="pv", name="pv")
                ps_cs = pscs.tile([128, QB], F32, tag="cs", name="cs")
                for kt in range(KT):
                    ps_s = psmm.tile([128, 512], F32, tag="mm", name="mm")[:, :QB]
                    for ct in range(CT):
                        nc.tensor.matmul(
                            ps_s,
                            k_t[kt][:, ct, :],
                            q_t[b][:, ct, :],
                            start=(ct == 0),
                            stop=(ct == CT - 1),
                        )
                    e_t = work.tile([128, QB], MMDT, tag="exp", name="exp")
                    nc.scalar.activation(e_t[:], ps_s, AF.Exp, scale=SCALE)
                    _flush_pend()
                    pend.append((b, kt, e_t, ps_pv, ps_cs))

            def conv9(img, wgt_sb, dil, b):
                """3x3 conv (pad == dil) for output rows b*RB..b*RB+RB-1."""
                ps = psmm.tile([128, 512], F32, tag="mm", name="mm")[:, :QB]
                for t in range(9):
                    ky, kx = divmod(t, 3)
                    rhs = img[
                        :,
                        b * RB + dil * ky : b * RB + dil * ky + RB,
                        dil * kx : dil * kx + W,
                    ]
                    nc.tensor.matmul(
                        ps[:],
                        wgt_sb[:, t, :],
                        rhs,
                        start=(t == 0),
                        stop=(t == 8),
                    )
                return ps

            def conv1_blk(b):
                # v2 = v + lrelu(conv3x3(p))
                ps = conv9(p_img, wo_sb, 1, b)
                lr = work.tile([128, QB], F32, tag="lr", name="lr")
                nc.scalar.mul(lr[:], ps[:], LRELU_SLOPE)
                nc.vector.tensor_max(lr[:], lr[:], ps[:])
                nc.vector.tensor_add(
                    v2_img[:, 2 + b * RB : 2 + (b + 1) * RB, 2 : 2 + W],
                    _r3(lr[:]),
                    vl_sb[:, b * RB : (b + 1) * RB, :],
                )

            def conv2_blk(b):
                # f = lrelu(dconv3x3_d2(v2))
                ps = conv9(v2_img, wf1_sb, 2, b)
                lr = work.tile([128, QB], F32, tag="lr", name="lr")
                nc.scalar.mul(lr[:], ps[:], LRELU_SLOPE)
                nc.vector.tensor_max(
                    f_img[:, 1 + b * RB : 1 + (b + 1) * RB, 1 : 1 + W],
                    _r3(lr[:]),
                    _r3(ps[:]),
                )

            def conv3_blk(b):
                # out = v2 + lrelu(conv3x3(f))
                ps = conv9(f_img, wf2_sb, 1, b)
                lr = work.tile([128, QB], F32, tag="lr", name="lr")
                nc.scalar.mul(lr[:], ps[:], LRELU_SLOPE)
                nc.vector.tensor_max(lr[:], lr[:], ps[:])
                nc.vector.tensor_add(
                    out_sb[:, b * RB : (b + 1) * RB, :],
                    _r3(lr[:]),
                    _f32(v2_img[:, 2 + b * RB : 2 + (b + 1) * RB, 2 : 2 + W]),
                )
                nc.sync.dma_start(
                    out_d[:, b * RB : (b + 1) * RB, :],
                    out_sb[:, b * RB : (b + 1) * RB, :],
                )

            # ---- interleaved schedule: conv block b_i only needs image
            # rows from earlier-emitted producers, so the PE can flow from
            # attention into convs without a phase barrier.
            attn_qb(0)
            attn_qb(1)
            attn_qb(2)
            conv1_blk(0)
            attn_qb(3)
            conv1_blk(1)
            conv2_blk(0)
            _flush_pend()
            conv1_blk(2)
            conv2_blk(1)
            conv3_blk(0)
            conv1_blk(3)
            conv2_blk(2)
            conv3_blk(1)
            conv2_blk(3)
            conv3_blk(2)
            conv3_blk(3)

    nc.finalize()
    return nc


def _r0(zer, img):
    """Slice of the zeros staging tile matching img's free size."""
    n = img.shape[1] * img.shape[2]
    return zer[:, :n].rearrange("p (r w) -> p r w", w=img.shape[2])


_NC_CACHE = []


def _get_nc():
    if not _NC_CACHE:
        _NC_CACHE.append(build_nc())
    return _NC_CACHE[0]


def _prep_core_inputs(k, q, v, wv_t, wo_t, wf1_t, wf2_t, n, r0):
    kn = k[n].reshape(CT, 128, KT, 128).transpose(1, 2, 0, 3)  # [128, kt, ct, 128]
    qn = (
        q[n]
        .reshape(CT, 128, H, W)[:, :, r0 : r0 + RQ, :]
        .reshape(CT, 128, NB, QB)
        .transpose(1, 2, 0, 3)
    )  # [128, qb, ct, QB]
    vn = v[n].reshape(D, HW)
    vl = v[n][:, r0 : r0 + RQ, :]
    return {
        "kk": np.ascontiguousarray(kn),
        "qq": np.ascontiguousarray(qn),
        "vv": np.ascontiguousarray(vn),
        "vloc": np.ascontiguousarray(vl),
        "wv": wv_t,
        "wo": wo_t,
        "wf1": wf1_t,
        "wf2": wf2_t,
    }


def kernel(k, q, v, w_value, w_out, w_ff1, w_ff2, _trace=False, _trace_kwargs=None):
    k = np.ascontiguousarray(np.asarray(k, dtype=np.float32))
    q = np.ascontiguousarray(np.asarray(q, dtype=np.float32))
    v = np.ascontiguousarray(np.asarray(v, dtype=np.float32))
    w_value = np.asarray(w_value, dtype=np.float32)
    w_out = np.asarray(w_out, dtype=np.float32)
    w_ff1 = np.asarray(w_ff1, dtype=np.float32)
    w_ff2 = np.asarray(w_ff2, dtype=np.float32)

    # per-tap transposed weights: [c_in, tap, c_out]
    wv_t = np.ascontiguousarray(w_value[:, :, 0, 0].T)
    wo_t = np.ascontiguousarray(w_out.transpose(1, 2, 3, 0).reshape(D, 9, D))
    wf1_t = np.ascontiguousarray(w_ff1.transpose(1, 2, 3, 0).reshape(D, 9, D))
    wf2_t = np.ascontiguousarray(w_ff2.transpose(1, 2, 3, 0).reshape(D, 9, D))

    in_maps = []
    for core in range(8):
        n, half = divmod(core, 2)
        r0 = 0 if half == 0 else H - RQ  # 0 or 20
        in_maps.append(_prep_core_inputs(k, q, v, wv_t, wo_t, wf1_t, wf2_t, n, r0))

    nc = _get_nc()
    kwargs = {}
    if _trace:
        kwargs = {"trace": True, **(_trace_kwargs or {})}
    res = run_bass_kernel_spmd(nc, in_maps, core_ids=list(range(8)), **kwargs)

    out = np.empty((N_BATCH, D, H, W), dtype=np.float32)
    for core in range(8):
        n, half = divmod(core, 2)
        local = res.results[core]["out"]  # [128, RQ, W]
        if half == 0:
            out[n, :, 0:24, :] = local[:, 0:24, :]
        else:
            out[n, :, 24:48, :] = local[:, RQ - 24 :, :]
    if _trace:
        return out, res
    return out


# revision 12
# speedup vs baseline: 1.3039x; 1.0542x over previous
"""Bass/Tile Trainium2 kernel for nn_CrossAttention (8 NeuronCores).

Sharding: data-parallel over batch N=4, and each sample's query positions
are split across 2 cores (rows 0..23 / 24..47 of the 48x64 image, plus a
4-row halo so the conv stack after attention needs no communication).

Per-core pipeline (uniform across cores; all asymmetry handled on host):
  valT = (w_value @ v)^T                       [kpos, d] tiles
  scores[k, q] = k^T q                         PE, fp32r
  e = exp(scores / sqrt(384))                  ACT, PSUM->SBUF (fp32r out)
  pv[d, q]  = val @ e                          PE accumulation over k-tiles
  cs[q]     = ones^T e  (broadcast to 128p)    PE accumulation over k-tiles
  p = pv / cs                                  DVE (reciprocal + mul)
  v2 = v + lrelu(conv3x3(p))                   PE + DVE
  f  = lrelu(dconv3x3_d2(v2))                  PE + DVE
  out = v2 + lrelu(conv3x3(f))                 PE + DVE

fp32r note: the PE runs fp32r (full-rate single-pass fp32) only when every
matmul operand was *produced* as fp32r by a compute instruction, so DMA'd
tensors are bounced through a rounding copy and on-chip producers (ACT exp,
DVE image writes) emit fp32r directly.
"""

import math

import numpy as np

import concourse.bass as bass
import concourse.mybir as mybir
import concourse.tile as tile
from concourse import bacc
from concourse.bass_utils import run_bass_kernel_spmd

F32 = mybir.dt.float32
F32R = mybir.dt.float32r
BF16 = mybir.dt.bfloat16
AF = mybir.ActivationFunctionType
ALU = mybir.AluOpType

# Problem constants (hardcoded per the contract)
N_BATCH = 4
C = 384            # kq channels
D = 128            # hidden
H, W = 48, 64
HW = H * W         # 3072
CT = C // 128      # 3 contraction tiles
KT = HW // 128     # 24 key-position tiles
RQ = 28            # local rows per core (24 output + 4 halo)
Q = RQ * W         # 1792 query positions per core
RB = 7             # image rows per block
NB = RQ // RB      # 4 blocks
QB = RB * W        # 448 block free size
SCALE = 1.0 / math.sqrt(C)
LRELU_SLOPE = 0.2

USE_F32R = True    # run the big matmuls in fp32r (full-rate fp32)

MMDT = F32R if USE_F32R else F32


def _r3(ap):
    """[128, QB] -> [128, RB, W] view."""
    return ap.rearrange("p (r w) -> p r w", w=W)


def _f32(ap):
    """Read an fp32r tile as plain fp32 (same bits) for DVE/ACT consumers."""
    return ap.bitcast(F32) if ap.dtype == F32R else ap


def build_nc():
    nc = bacc.Bacc(None, target_bir_lowering=False)

    kk = nc.dram_tensor("kk", [128, KT, CT, 128], MMDT, kind="ExternalInput")
    qq = nc.dram_tensor("qq", [128, NB, CT, QB], MMDT, kind="ExternalInput")
    vv = nc.dram_tensor("vv", [128, HW], F32, kind="ExternalInput")
    vloc = nc.dram_tensor("vloc", [128, RQ, W], F32, kind="ExternalInput")
    wv = nc.dram_tensor("wv", [128, 128], F32, kind="ExternalInput")
    wo = nc.dram_tensor("wo", [128, 9, 128], MMDT, kind="ExternalInput")
    wf1 = nc.dram_tensor("wf1", [128, 9, 128], MMDT, kind="ExternalInput")
    wf2 = nc.dram_tensor("wf2", [128, 9, 128], MMDT, kind="ExternalInput")
    out_d = nc.dram_tensor("out", [128, RQ, W], F32, kind="ExternalOutput")

    with tile.TileContext(nc) as tc:
        with (
            tc.tile_pool(name="io", bufs=1) as io,
            tc.tile_pool(name="work", bufs=4) as work,
            tc.tile_pool(name="psmm", bufs=4, space="PSUM") as psmm,
            tc.tile_pool(name="pspv", bufs=2, space="PSUM") as pspv,
            tc.tile_pool(name="pscs", bufs=2, space="PSUM") as pscs,
        ):
            # persistent SBUF tensors (matmul operands in MMDT)
            k_t = [io.tile([128, CT, 128], MMDT, tag=f"k{kt}", name=f"k{kt}") for kt in range(KT)]
            q_t = [io.tile([128, CT, QB], MMDT, tag=f"q{b}", name=f"q{b}") for b in range(NB)]
            v_sb = io.tile([128, HW], F32, tag="v", name="v")
            vl_sb = io.tile([128, RQ, W], F32, tag="vl", name="vl")
            wv_sb = io.tile([128, 128], F32, tag="wv", name="wv")
            wo_sb = io.tile([128, 9, 128], MMDT, tag="wo", name="wo")
            wf1_sb = io.tile([128, 9, 128], MMDT, tag="wf1", name="wf1")
            wf2_sb = io.tile([128, 9, 128], MMDT, tag="wf2", name="wf2")
            ones_sb = io.tile([128, 128], BF16, tag="ones", name="ones")
            valT = [io.tile([128, 512], BF16, tag=f"vt{j}", name=f"vt{j}") for j in range(KT // 4)]
            p_img = io.tile([128, RQ + 2, W + 2], MMDT, tag="p_img", name="p_img")
            v2_img = io.tile([128, RQ + 4, W + 4], MMDT, tag="v2_img", name="v2_img")
            f_img = io.tile([128, RQ + 2, W + 2], MMDT, tag="f_img", name="f_img")
            out_sb = io.tile([128, RQ, W], F32, tag="out_sb", name="out_sb")

            # input DMAs; v + wv first (they feed valT, the first PE work),
            # then q block 0 + k tiles so scores can start ASAP
            nc.sync.dma_start(wv_sb[:], wv[:])
            nc.sync.dma_start(v_sb[:, 0:512], vv[:, 0:512])
            for ct in range(CT):
                nc.sync.dma_start(q_t[0][:, ct, :], qq[:, 0, ct, :])
            for kt in range(KT):
                nc.sync.dma_start(k_t[kt][:], kk[:, kt, :, :])
                if kt < KT // 4 - 1:
                    j = kt + 1
                    nc.sync.dma_start(
                        v_sb[:, j * 512 : (j + 1) * 512],
                        vv[:, j * 512 : (j + 1) * 512],
                    )
            for b in range(1, NB):
                nc.sync.dma_start(q_t[b][:], qq[:, b, :, :])
            nc.sync.dma_start(vl_sb[:], vloc[:])
            nc.sync.dma_start(wo_sb[:], wo[:])
            nc.sync.dma_start(wf1_sb[:], wf1[:])
            nc.sync.dma_start(wf2_sb[:], wf2[:])

            # constants / zero borders, produced by compute ops so the
            # fp32r verifier sees rounded producers
            zer = io.tile([128, (RQ + 4) * (W + 4)], F32, tag="zer", name="zer")
            nc.vector.memset(zer[:], 0.0)
            one = io.tile([128, 128], F32, tag="one", name="one")
            nc.vector.memset(one[:], 1.0)
            nc.vector.tensor_copy(ones_sb[:], one[:])
            nc.vector.tensor_copy(p_img[:], _r0(zer, p_img))
            nc.vector.tensor_copy(v2_img[:], _r0(zer, v2_img))
            nc.vector.tensor_copy(f_img[:], _r0(zer, f_img))

            # valT[kpos, d] = sum_c v[c, kpos] * w_value[d, c]  (plain fp32:
            # N=128 gets no fp32r speedup and fp32 avoids rounding v/wv)
            def valT_group(j):
                ps = psmm.tile([128, 512], F32, tag="mm", name="mm")
                for i in range(4):
                    kt = j * 4 + i
                    nc.tensor.matmul(
                        ps[:, i * 128 : (i + 1) * 128],
                        v_sb[:, kt * 128 : (kt + 1) * 128],
                        wv_sb[:],
                        start=True,
                        stop=True,
                    )
                nc.any.tensor_copy(valT[j][:], ps[:])

            valT_group(0)

            # ---- stage emitters --------------------------------------
            # software-pipelined attention: the pv/cs consumption of e_t(kt)
            # is emitted one k-tile late, so the PE never sits at the head of
            # its FIFO waiting for the exp of the tile it just produced.
            pend = []

            def _flush_pend():
                if not pend:
                    return
                b, kt, e_t, ps_pv, ps_cs = pend.pop()
                nc.tensor.matmul(
                    ps_pv[:],
                    valT[kt // 4][:, (kt % 4) * 128 : (kt % 4 + 1) * 128],
                    e_t[:],
                    start=(kt == 0),
                    stop=(kt == KT - 1),
                )
                nc.tensor.matmul(
                    ps_cs[:],
                    ones_sb[:],
                    e_t[:],
                    start=(kt == 0),
                    stop=(kt == KT - 1),
                )
                if kt == KT - 1:
                    pv_sb = work.tile([128, QB], F32, tag="pvsb", name="pvsb")
                    nc.any.tensor_copy(pv_sb[:], ps_pv[:])
                    cs_sb = work.tile([128, QB], F32, tag="cssb", name="cssb")
                    nc.any.tensor_copy(cs_sb[:], ps_cs[:])
                    rcp = work.tile([128, QB], F32, tag="rcp", name="rcp")
                    nc.vector.reciprocal_approx_fast(rcp[:], cs_sb[:])
                    nc.vector.tensor_mul(
                        p_img[:, 1 + b * RB : 1 + (b + 1) * RB, 1 : 1 + W],
                        _r3(pv_sb[:]),
                        _r3(rcp[:]),
                    )

            def attn_qb(b):
                """scores -> exp; pv/cs emission trails by one k-tile."""
                ps_pv = pspv.tile([128, QB], F32, tag="pv", name="pv")
                ps_cs = pscs.tile([128, QB], F32, tag="cs", name="cs")
                for kt in range(KT):
                    ps_s = psmm.tile([128, 512], F32, tag="mm", name="mm")[:, :QB]
                    for ct in range(CT):
                        nc.tensor.matmul(
                            ps_s,
                            k_t[kt][:, ct, :],
                            q_t[b][:, ct, :],
                            start=(ct == 0),
                            stop=(ct == CT - 1),
                        )
                    e_t = work.tile([128, QB], BF16, tag="exp", name="exp")
                    nc.scalar.activation(e_t[:], ps_s, AF.Exp, scale=SCALE)
                    _flush_pend()
                    pend.append((b, kt, e_t, ps_pv, ps_cs))
                    if b == 0 and kt % 4 == 3 and kt < 4 * (KT // 4 - 1):
                        valT_group((kt + 1) // 4)

            def conv9(img, wgt_sb, dil, r0, nr):
                """3x3 conv (pad == dil) for output rows r0..r0+nr-1."""
                ps = psmm.tile([128, 512], F32, tag="mm", name="mm")[:, : nr * W]
                for t in range(9):
                    ky, kx = divmod(t, 3)
                    rhs = img[
                        :,
                        r0 + dil * ky : r0 + dil * ky + nr,
                        dil * kx : dil * kx + W,
                    ]
                    nc.tensor.matmul(
                        ps[:],
                        wgt_sb[:, t, :],
                        rhs,
                        start=(t == 0),
                        stop=(t == 8),
                    )
                return ps

            def _rr(ap, nr):
                return ap.rearrange("p (r w) -> p r w", w=W)

            def conv1_blk(r0, nr):
                # v2 = v + lrelu(conv3x3(p))
                ps = conv9(p_img, wo_sb, 1, r0, nr)
                lr = work.tile([128, QB], F32, tag="lr", name="lr")[:, : nr * W]
                nc.scalar.mul(lr, ps[:], LRELU_SLOPE)
                nc.vector.tensor_max(lr, lr, ps[:])
                nc.vector.tensor_add(
                    v2_img[:, 2 + r0 : 2 + r0 + nr, 2 : 2 + W],
                    _rr(lr, nr),
                    vl_sb[:, r0 : r0 + nr, :],
                )

            def conv2_blk(r0, nr):
                # f = lrelu(dconv3x3_d2(v2))
                ps = conv9(v2_img, wf1_sb, 2, r0, nr)
                lr = work.tile([128, QB], F32, tag="lr", name="lr")[:, : nr * W]
                nc.scalar.mul(lr, ps[:], LRELU_SLOPE)
                nc.vector.tensor_max(
                    f_img[:, 1 + r0 : 1 + r0 + nr, 1 : 1 + W],
                    _rr(lr, nr),
                    _rr(ps[:], nr),
                )

            def conv3_blk(r0, nr):
                # out = v2 + lrelu(conv3x3(f))
                ps = conv9(f_img, wf2_sb, 1, r0, nr)
                lr = work.tile([128, QB], F32, tag="lr", name="lr")[:, : nr * W]
                nc.scalar.mul(lr, ps[:], LRELU_SLOPE)
                nc.vector.tensor_max(lr, lr, ps[:])
                nc.vector.tensor_add(
                    out_sb[:, r0 : r0 + nr, :],
                    _rr(lr, nr),
                    _f32(v2_img[:, 2 + r0 : 2 + r0 + nr, 2 : 2 + W]),
                )
                nc.sync.dma_start(
                    out_d[:, r0 : r0 + nr, :],
                    out_sb[:, r0 : r0 + nr, :],
                )

            # ---- interleaved schedule: conv block b_i only needs image
            # rows from earlier-emitted producers, so the PE can flow from
            # attention into convs without a phase barrier.
            attn_qb(0)
            attn_qb(1)
            attn_qb(2)
            conv1_blk(0, 7)
            attn_qb(3)
            conv1_blk(7, 7)
            conv2_blk(0, 7)
            _flush_pend()
            conv1_blk(14, 7)
            conv2_blk(7, 7)
            conv3_blk(0, 7)
            conv1_blk(21, 4)
            conv1_blk(25, 3)
            conv2_blk(14, 7)
            conv3_blk(7, 7)
            conv2_blk(21, 4)
            conv2_blk(25, 3)
            conv3_blk(14, 7)
            conv3_blk(21, 4)
            conv3_blk(25, 3)

    nc.finalize()
    return nc


def _r0(zer, img):
    """Slice of the zeros staging tile matching img's free size."""
    n = img.shape[1] * img.shape[2]
    return zer[:, :n].rearrange("p (r w) -> p r w", w=img.shape[2])


_NC_CACHE = []


def _get_nc():
    if not _NC_CACHE:
        _NC_CACHE.append(build_nc())
    return _NC_CACHE[0]


def _prep_core_inputs(k, q, v, wv_t, wo_t, wf1_t, wf2_t, n, r0):
    kn = k[n].reshape(CT, 128, KT, 128).transpose(1, 2, 0, 3)  # [128, kt, ct, 128]
    qn = (
        q[n]
        .reshape(CT, 128, H, W)[:, :, r0 : r0 + RQ, :]
        .reshape(CT, 128, NB, QB)
        .transpose(1, 2, 0, 3)
    )  # [128, qb, ct, QB]
    vn = v[n].reshape(D, HW)
    vl = v[n][:, r0 : r0 + RQ, :]
    return {
        "kk": np.ascontiguousarray(kn),
        "qq": np.ascontiguousarray(qn),
        "vv": np.ascontiguousarray(vn),
        "vloc": np.ascontiguousarray(vl),
        "wv": wv_t,
        "wo": wo_t,
        "wf1": wf1_t,
        "wf2": wf2_t,
    }


def kernel(k, q, v, w_value, w_out, w_ff1, w_ff2, _trace=False, _trace_kwargs=None):
    k = np.ascontiguousarray(np.asarray(k, dtype=np.float32))
    q = np.ascontiguousarray(np.asarray(q, dtype=np.float32))
    v = np.ascontiguousarray(np.asarray(v, dtype=np.float32))
    w_value = np.asarray(w_value, dtype=np.float32)
    w_out = np.asarray(w_out, dtype=np.float32)
    w_ff1 = np.asarray(w_ff1, dtype=np.float32)
    w_ff2 = np.asarray(w_ff2, dtype=np.float32)

    # per-tap transposed weights: [c_in, tap, c_out]
    wv_t = np.ascontiguousarray(w_value[:, :, 0, 0].T)
    wo_t = np.ascontiguousarray(w_out.transpose(1, 2, 3, 0).reshape(D, 9, D))
    wf1_t = np.ascontiguousarray(w_ff1.transpose(1, 2, 3, 0).reshape(D, 9, D))
    wf2_t = np.ascontiguousarray(w_ff2.transpose(1, 2, 3, 0).reshape(D, 9, D))

    in_maps = []
    for core in range(8):
        n, half = divmod(core, 2)
        r0 = 0 if half == 0 else H - RQ  # 0 or 20
        in_maps.append(_prep_core_inputs(k, q, v, wv_t, wo_t, wf1_t, wf2_t, n, r0))

    nc = _get_nc()
    kwargs = {}
    if _trace:
        kwargs = {"trace": True, **(_trace_kwargs or {})}
    res = run_bass_kernel_spmd(nc, in_maps, core_ids=list(range(8)), **kwargs)

    out = np.empty((N_BATCH, D, H, W), dtype=np.float32)
    for core in range(8):
        n, half = divmod(core, 2)
        local = res.results[core]["out"]  # [128, RQ, W]
        if half == 0:
            out[n, :, 0:24, :] = local[:, 0:24, :]
        else:
            out[n, :, 24:48, :] = local[:, RQ - 24 :, :]
    if _trace:
        return out, res
    return out
